# revision 44
# baseline (speedup 1.0000x reference)
"""Trainium2 Bass kernel for ConditionalGraphGenerator (GCN message passing).

Contract: kernel(**inputs) takes the FULL unsharded inputs (numpy arrays,
keys as in reference.setup_inputs()) and returns the FULL [256, 512, 2]
float32 output. Internally shards the batch dim across 8 NeuronCores
(pure data parallel, 32 batches per core).

Math (same factorization as the previous revision): the symmetric
normalization and node-validity masking fold into one adjacency on host:
Adj = s.(A+diag(m)).s with s = m*deg^-1/2, so Adj is exactly zero outside
the leading [nn, nn] block (nn = num_nodes). Per batch the device computes
  R1 = relu(Adj @ L1)           L1 = layout@w_gcn1 (host)
  W2 = (R1 @ w_gcn2)            layout-fixing transposing MMs
  R2 = relu(Adj @ W2)
restricted to the leading ceil(nn/128) node tiles and nn output columns.
The output projection, noise path, bias adds, and final mask run on host:
out = m.(R2^T wout[:H] + cc),  cc = relu(z@w_noise+b_noise)@w_out[H:]+b_out.
(gcn biases are zero in the graded inputs; a slower exact bias path exists.)

This revision restructures the schedule around the measured bottlenecks of
the previous one (DMA-trigger sequencer time on the busy engines, clock
p-state resets from tensor-engine gaps, ACT/DVE evacuation imbalance, and
padded-tile DMA traffic):
- adjacency and L1 ship as ONE merged fp8 dram tensor per 4-batch group,
  packed raggedly by each slot's exact tile count (kteff), so a group costs
  one DMA trigger and ships no padded node tiles; compute/evacuation column
  counts are the exact per-slot node counts (only the shipped adjacency
  width is 32-aligned - the dual-fp8 ldweights path needs aligned tile
  pitch);
- every DMA trigger (8 group loads + the stores) issues from the otherwise
  idle sync engine's hardware DGE with 4 groups of lookahead, stores
  emitted ahead of the next load on the queue; gpsimd (which cannot touch
  PSUM on TRN2) only loads the w_gcn2 const and does startup memsets;
- the three PSUM evacuations per batch (R1, W2, R2) are spread across
  ACT/DVE by a build-time greedy balancer using measured per-op costs
  (ACT 0.87ns/col + 230, DVE 1.04ns/col + 135), with strict alternation
  for the drain group so its evacuations run in parallel;
- R2 ships raggedly packed by exact column count, per-group mid-kernel and
  per-slot for the last two groups so the final store is small;
- groups are processed in ascending size order, which measures fastest
  (the tensor engine stays continuously fed through the body - matmul
  durations confirm full clock; other orders measured up to 20% slower
  clocks). Steady state is tensor/evac co-paced at ~770ns/slot; the ~8us
  walrus NEFF epilogue (serialized per-engine semaphore zeroing) and
  ~2.5us first-DMA latency are fixed overheads outside kernel control.
"""

import sys

if "/opt/trn_rl_repo" not in sys.path:
    sys.path.insert(0, "/opt/trn_rl_repo")

import ml_dtypes
import numpy as np

import concourse.bass as bass
import concourse.tile as tile
from concourse import bacc, mybir
from concourse.bass_utils import run_bass_kernel_spmd

B, N, H, LAT, OUT = 256, 512, 128, 128, 2
NCORES = 8
BPC = B // NCORES          # batches per core = 32
PT = N // 128              # 4 node tiles max
GRP = 4                    # batches per group (one input / output DMA each)
NGRP = BPC // GRP          # 8 groups per core

F32 = mybir.dt.float32
BF16 = mybir.dt.bfloat16
F8 = mybir.dt.float8e4
AF = mybir.ActivationFunctionType
ALU = mybir.AluOpType
DR = mybir.MatmulPerfMode.DoubleRow
NPBF16 = ml_dtypes.bfloat16
NPF8 = mybir.dt.np(F8)

# power-of-2 prescales (exact; folded back out in the evacuation ops)
EA = 2.0 ** 7              # adjacency
EC = 2.0 ** 5              # L1
ER1 = 2.0 ** 8             # R1 (fp8 intermediate)
ER2 = 2.0 ** 11            # W2 (fp8 intermediate)
ER3 = 2.0 ** 13            # R2 (fp8 output shipped to host)

# group processing order: small first (fast pipeline fill), smallest
# last (fast drain); the big groups stream through the middle where the
# DMA queue and tensor engine are both saturated.
ORDER = (0, 1, 2, 3, 4, 5, 6, 7)
LOOKAHEAD = 4

_CACHE = {}


def _evac_plan(order_slots, cfg):
    """Greedy-assign the PSUM evacuations to ACT/DVE by running-total cost
    (measured-ns models: ACT 0.87/col + 230, DVE 1.04/col + 135)."""
    kts, sges, nouts, kteffs, gno, noff, hasb = cfg
    load = {"act": 0.0, "dve": 0.0}
    assign = {}
    items = []
    for s in order_slots:
        items.append(("r1", s, nouts[s]))
        items.append(("w2", s, kteffs[s] * 128))
        items.append(("r2", s, nouts[s]))
    tail = set(order_slots[-GRP:])
    for ev, s, cols in items:
        if hasb and ev in ("r1", "r2"):
            assign[(ev, s)] = "act"
            load["act"] += 0.87 * cols + 230
            continue
        if ev == "r2" and s in tail:
            # strict alternation so the drain's evacuations run in parallel
            e = ("dve", "act")[s % 2]
        else:
            cost = {"act": 0.87 * cols + 230, "dve": 1.04 * cols + 135}
            e = min(("act", "dve"), key=lambda c: load[c] + cost[c])
        assign[(ev, s)] = e
        load[e] += {"act": 0.87, "dve": 1.04}[e] * cols + \
                   {"act": 230, "dve": 135}[e]
    return assign


def _build(cfg):
    """cfg = (kts, sges, nouts, kteffs, gno, noff, hasb): per-group max tile
    counts and shipped adjacency widths, per-slot exact output widths and
    effective tile counts, per-group ragged output width, per-slot ragged
    output column offset, nonzero-gcn-bias flag."""
    kts, sges, nouts, kteffs, gno, noff, hasb = cfg
    nc = bacc.Bacc("TRN2", target_bir_lowering=False, debug=False,
                   enable_asserts=False, num_devices=NCORES)

    # per-slot tile offset inside its group's merged tensor
    ktoff = [0] * BPC
    gkt = [0] * NGRP
    for g in range(NGRP):
        off = 0
        for bb in range(GRP):
            ktoff[g * GRP + bb] = off
            off += kteffs[g * GRP + bb]
        gkt[g] = off

    agl, r2o = [], []
    for g in range(NGRP):
        sge = sges[g]
        # agl_g[p, toff+u, 0:sge]   = (EA*Adj^T)[u*128+p, i] of its slot
        # agl_g[p, toff+u, sge:]    = (EC*L1)[u*128+p, h]
        agl.append(nc.dram_tensor(f"agl{g}", [128, gkt[g], sge + H], F8,
                                  kind="ExternalInput").ap())
        r2o.append(nc.dram_tensor(f"r2o{g}", [H, gno[g]], F8,
                                  kind="ExternalOutput").ap())
    wg2 = nc.dram_tensor("wg2", [H, H], BF16, kind="ExternalInput").ap()
    if hasb:
        b1s = nc.dram_tensor("b1s", [H, 1], F32, kind="ExternalInput").ap()
        b2s = nc.dram_tensor("b2s", [H, 1], F32, kind="ExternalInput").ap()

    order_slots = [ORDER[j] * GRP + bb for j in range(NGRP)
                   for bb in range(GRP)]
    ev_of = _evac_plan(order_slots, cfg)

    with tile.TileContext(nc) as tc:
        with tc.tile_pool(name="consts", bufs=1) as cpool, \
             tc.tile_pool(name="agl", bufs=7) as agl_pool, \
             tc.tile_pool(name="r1", bufs=3) as r1_pool, \
             tc.tile_pool(name="w2", bufs=3) as w2_pool, \
             tc.tile_pool(name="r2g", bufs=3) as r2g_pool, \
             tc.tile_pool(name="psR1", bufs=3, space="PSUM") as psR1_pool, \
             tc.tile_pool(name="psG", bufs=2, space="PSUM") as psG_pool, \
             tc.tile_pool(name="psR2", bufs=3, space="PSUM") as psR2_pool:

            agl_of, r1_of, w2_of, r2_of = {}, {}, {}, {}

            def dma_in(j):
                g = ORDER[j]
                AGL = agl_pool.tile([128, gkt[g], sges[g] + H], F8, tag="agl")
                nc.sync.dma_start(AGL[:], agl[g][:])
                agl_of[g] = AGL

            for j in range(LOOKAHEAD):
                dma_in(j)

            # w_gcn2 const on gpsimd's queue so sync stays free for groups
            WG2 = cpool.tile([H, H], BF16)
            nc.gpsimd.dma_start(WG2[:], wg2[:])
            if hasb:
                B1S = cpool.tile([H, 1], F32)
                nc.gpsimd.dma_start(B1S[:], b1s[:])
                B2S = cpool.tile([H, 1], F32)
                nc.gpsimd.dma_start(B2S[:], b2s[:])

            # stale R1 columns beyond a slot's exact width feed provably
            # cancelled products; memset once so they are finite fp8
            for _z in range(3):
                R1Z = r1_pool.tile([H, N], F8, tag="r1t")
                nc.gpsimd.memset(R1Z[:], 0)

            def adj_pass(psum, lhs3, lbase, AGL, abase, kt, no):
                """psum += sum_u lhs3[:,lbase+u,:]^T @ adj[:,abase+u,:no]."""
                ndr, rem = kt // 2, kt % 2
                for u in range(ndr):
                    nc.tensor.matmul(
                        psum, lhs3[:, lbase + 2 * u:lbase + 2 * u + 2, :],
                        AGL[:, abase + 2 * u:abase + 2 * u + 2, :no],
                        start=(u == 0), stop=(rem == 0 and u == ndr - 1),
                        perf_mode=DR)
                if rem:
                    nc.tensor.matmul(
                        psum, lhs3[:, lbase + kt - 1, :],
                        AGL[:, abase + kt - 1, :no],
                        start=(ndr == 0), stop=True)

            def evac(which, s, dst, src, scale):
                e = ev_of[(which, s)]
                if which == "w2":
                    if e == "act":
                        nc.scalar.activation(dst, src, AF.Copy, scale=scale)
                    else:
                        nc.vector.tensor_scalar_mul(dst, src, scale)
                elif hasb:
                    bias = B1S if which == "r1" else B2S
                    nc.scalar.activation(dst, src, AF.Relu, bias=bias[:],
                                         scale=scale)
                elif e == "act":
                    nc.scalar.activation(dst, src, AF.Relu, scale=scale)
                else:
                    nc.vector.tensor_scalar(dst, src, scale, 0.0,
                                            ALU.mult, ALU.max)

            for t in range(BPC + 4):
                if t % 4 == 0 and t // 4 + LOOKAHEAD < NGRP:
                    dma_in(t // 4 + LOOKAHEAD)

                if t < BPC:
                    # pass1: psR1 = (EA*EC) * L1^T Adj^T  over kteff tiles
                    s = order_slots[t]
                    g = s // GRP
                    no = nouts[s]
                    sge = sges[g]
                    AGL = agl_of[g]
                    psR1 = psR1_pool.tile([H, N], F32, tag="psr1")
                    adj_pass(psR1[:, :no], AGL[:, :, sge:sge + H],
                             ktoff[s], AGL, ktoff[s], kteffs[s], no)
                    R1T = r1_pool.tile([H, N], F8, tag="r1t")
                    evac("r1", s, R1T[:, :no], psR1[:, :no], ER1 / (EA * EC))
                    r1_of[s] = R1T

                if 0 <= t - 2 < BPC:
                    # G: psG[:, u, :] = ER1 * (R1 @ w2) tile u (layout fix)
                    s1 = order_slots[t - 2]
                    kt1 = kteffs[s1]
                    R1T = r1_of.pop(s1)
                    psG = psG_pool.tile([128, PT, H], F32, tag="psg")
                    for u in range(kt1):
                        nc.tensor.matmul(
                            psG[:, u, :], R1T[:, bass.ts(u, 128)],
                            WG2[:], start=True, stop=True)
                    W2T = w2_pool.tile([128, PT, H], F8, tag="w2t")
                    evac("w2", s1, W2T[:, :kt1, :], psG[:, :kt1, :],
                         ER2 / ER1)
                    w2_of[s1] = W2T

                if 0 <= t - 4 < BPC:
                    # pass2 + fp8 R2 evacuation into the ragged group tile
                    s2 = order_slots[t - 4]
                    g2, bb2 = divmod(s2, GRP)
                    no2 = nouts[s2]
                    W2T = w2_of.pop(s2)
                    psR2 = psR2_pool.tile([H, N], F32, tag="psr2")
                    adj_pass(psR2[:, :no2], W2T, 0,
                             agl_of[g2], ktoff[s2], kteffs[s2], no2)
                    if bb2 == 0:
                        R2G = r2g_pool.tile([H, gno[g2]], F8, tag="r2g")
                        r2_of[g2] = R2G
                    R2G = r2_of[g2]
                    o2 = noff[s2]
                    if s2 == order_slots[-1] and not hasb and no2 >= 32:
                        # the very last evacuation+store is the exposed
                        # tail chain: evacuate the two halves into
                        # DIFFERENT tiles (same-tile writers serialize in
                        # Tile) on ACT and DVE, and ship them from two
                        # DMA queues so both chains run fully in parallel
                        hf = no2 // 2
                        sc = ER3 / (EA * ER2)
                        nc.scalar.activation(R2G[:, o2:o2 + hf],
                                             psR2[:, :hf], AF.Relu,
                                             scale=sc)
                        S2T = cpool.tile([H, 256], F8)
                        nc.vector.tensor_scalar(S2T[:, :no2 - hf],
                                                psR2[:, hf:no2], sc, 0.0,
                                                ALU.mult, ALU.max)
                        nc.sync.dma_start(r2o[g2][:, o2:o2 + hf],
                                          R2G[:, o2:o2 + hf])
                        nc.scalar.dma_start(r2o[g2][:, o2 + hf:o2 + no2],
                                            S2T[:, :no2 - hf])
                    else:
                        evac("r2", s2, R2G[:, o2:o2 + no2],
                             psR2[:, :no2], ER3 / (EA * ER2))
                        if (t - 4) // 4 >= NGRP - 2:
                            # drain groups ship per-slot so the final
                            # store overlaps the remaining evacuations
                            nc.sync.dma_start(
                                r2o[g2][:, o2:o2 + no2],
                                R2G[:, o2:o2 + no2])
                        elif bb2 == GRP - 1:
                            nc.sync.dma_start(r2o[g2][:], R2G[:])

    nc.compile()
    return nc


def _get_nc(cfg):
    if cfg not in _CACHE:
        _CACHE[cfg] = _build(cfg)
    return _CACHE[cfg]


def _plan(num_nodes, hasb):
    """Sort batches ascending by node count, deal round-robin to cores."""
    nn_ = np.asarray(num_nodes)
    order = np.argsort(nn_, kind="stable")                  # ascending
    # slot k of core c runs batch order[k*NCORES + c]
    assign = order.reshape(BPC, NCORES)                     # [slot, core]
    # exact per-slot output width
    nn_slot = nn_[assign].max(axis=1)                       # [BPC]
    nouts = tuple(int(min(int(v), N)) for v in nn_slot)
    # effective contraction tile count per slot (zero rows beyond nn)
    kteffs = tuple(-(-no // 128) for no in nouts)
    # per-group shipped adjacency column count (max slot width in group,
    # 32-aligned so the merged tensor's tile pitch keeps the dual-fp8
    # ldweights address restrictions satisfied)
    sges = tuple(int(-(-max(nouts[g * GRP:(g + 1) * GRP]) // 32) * 32)
                 for g in range(NGRP))
    kts = tuple(int(max(kteffs[g * GRP:(g + 1) * GRP]))
                for g in range(NGRP))
    # ragged output packing: per-slot column offset, per-group total
    noff, gno = [0] * BPC, [0] * NGRP
    for g in range(NGRP):
        off = 0
        for bb in range(GRP):
            noff[g * GRP + bb] = off
            off += nouts[g * GRP + bb]
        gno[g] = off
    return assign, (kts, sges, nouts, kteffs, tuple(gno), tuple(noff),
                    bool(hasb))


def _host_prep(z, input_layout, adj_matrix, num_nodes,
               w_gcn1, b_gcn1, w_gcn2, b_gcn2,
               w_noise, b_noise, w_out, b_out):
    f32 = np.float32
    adj = np.asarray(adj_matrix, f32)
    layout = np.asarray(input_layout, f32)
    nn_ = np.asarray(num_nodes)
    m = (np.arange(N)[None, :] < nn_[:, None]).astype(f32)              # [B,N]

    hasb = bool(np.any(np.asarray(b_gcn1)) or np.any(np.asarray(b_gcn2)))
    assign, cfg = _plan(num_nodes, hasb)
    kts, sges, nouts, kteffs, gno, noff, _ = cfg

    # degree of the masked graph incl. self-loops (BLAS gemv), clamp at 1
    degr = np.matmul(adj, m[:, :, None])[:, :, 0] + m                   # [B,N]
    deg = np.maximum(m * degr, 1.0)
    s = (m / np.sqrt(deg)).astype(f32)                                  # [B,N]

    # Adj^T with normalization+mask folded: at[b,j,i] = s_j A[i,j] s_i (+diag)
    at = np.ascontiguousarray(adj.transpose(0, 2, 1))                   # [B,j,i]
    at *= (EA * s)[:, :, None]
    at *= s[:, None, :]
    idx = np.arange(N)
    at[:, idx, idx] += EA * s * s                                       # diag m/deg
    at8 = at.astype(NPF8)

    l1 = (layout @ (EC * np.asarray(w_gcn1, f32))).astype(NPF8)         # [B,N,H]

    ze = np.maximum(np.asarray(z, f32) @ np.asarray(w_noise, f32)
                    + np.asarray(b_noise, f32), 0.0)                    # [B,H]
    wout = np.asarray(w_out, f32)
    cc = (ze @ wout[H:] + np.asarray(b_out, f32)).astype(f32)           # [B,OUT]

    wg2 = np.ascontiguousarray(np.asarray(w_gcn2, f32)).astype(NPBF16)
    wouth = np.ascontiguousarray(wout[:H])                              # host side

    per_core = [{"wg2": wg2} for _ in range(NCORES)]
    if hasb:
        b1sv = (np.asarray(b_gcn1, f32) * ER1).reshape(H, 1).copy()
        b2sv = (np.asarray(b_gcn2, f32) * ER3).reshape(H, 1).copy()
        for c in range(NCORES):
            per_core[c]["b1s"] = b1sv
            per_core[c]["b2s"] = b2sv

    for g in range(NGRP):
        sge = sges[g]
        gkt = sum(kteffs[g * GRP + bb] for bb in range(GRP))
        for c in range(NCORES):
            ab = np.zeros((128, gkt, sge + H), NPF8)
            off = 0
            for bb in range(GRP):
                k = g * GRP + bb
                kte = kteffs[k]
                sj = 128 * kte
                b = int(assign[k, c])
                # at8[b, :sj, :sge] -> [kte,128,sge] -> [128,kte,sge]
                ab[:, off:off + kte, :sge] = (
                    at8[b, :sj, :sge].reshape(kte, 128, sge)
                    .transpose(1, 0, 2))
                ab[:, off:off + kte, sge:] = (
                    l1[b, :sj, :].reshape(kte, 128, H).transpose(1, 0, 2))
                off += kte
            per_core[c][f"agl{g}"] = ab
    return per_core, (cc, m, wouth, assign, cfg)


def _unpack(res, ctx):
    cc, m, wouth, assign, cfg = ctx
    kts, sges, nouts, kteffs, gno, noff, hasb = cfg
    ots = np.zeros((B, N, OUT), np.float32)
    inv_er3 = np.float32(1.0 / ER3)
    for g in range(NGRP):
        for c in range(NCORES):
            r2 = res.results[c][f"r2o{g}"].astype(np.float32)          # [H,gno]
            for bb in range(GRP):
                k = g * GRP + bb
                b = int(assign[k, c])
                no = nouts[k]
                o = noff[k]
                ots[b, :no, :] = (r2[:, o:o + no].T @ wouth) * inv_er3
    out = (ots + cc[:, None, :]) * m[:, :, None]
    return np.ascontiguousarray(out).astype(np.float32)


def kernel(**inputs):
    in_maps, ctx = _host_prep(**inputs)
    nc = _get_nc(ctx[4])
    res = run_bass_kernel_spmd(nc, in_maps, list(range(NCORES)))
    return _unpack(res, ctx)


# revision 46
# speedup vs baseline: 1.0009x; 1.0009x over previous
"""Trainium2 Bass kernel for ConditionalGraphGenerator (GCN message passing).

Contract: kernel(**inputs) takes the FULL unsharded inputs (numpy arrays,
keys as in reference.setup_inputs()) and returns the FULL [256, 512, 2]
float32 output. Internally shards the batch dim across 8 NeuronCores
(pure data parallel, 32 batches per core).

Math (same factorization as the previous revision): the symmetric
normalization and node-validity masking fold into one adjacency on host:
Adj = s.(A+diag(m)).s with s = m*deg^-1/2, so Adj is exactly zero outside
the leading [nn, nn] block (nn = num_nodes). Per batch the device computes
  R1 = relu(Adj @ L1)           L1 = layout@w_gcn1 (host)
  W2 = (R1 @ w_gcn2)            layout-fixing transposing MMs
  R2 = relu(Adj @ W2)
restricted to the leading ceil(nn/128) node tiles and nn output columns.
The output projection, noise path, bias adds, and final mask run on host:
out = m.(R2^T wout[:H] + cc),  cc = relu(z@w_noise+b_noise)@w_out[H:]+b_out.
(gcn biases are zero in the graded inputs; a slower exact bias path exists.)

This revision restructures the schedule around the measured bottlenecks of
the previous one (DMA-trigger sequencer time on the busy engines, clock
p-state resets from tensor-engine gaps, ACT/DVE evacuation imbalance, and
padded-tile DMA traffic):
- adjacency and L1 ship as ONE merged fp8 dram tensor per 4-batch group,
  packed raggedly by each slot's exact tile count (kteff), so a group costs
  one DMA trigger and ships no padded node tiles; compute/evacuation column
  counts are the exact per-slot node counts (only the shipped adjacency
  width is 32-aligned - the dual-fp8 ldweights path needs aligned tile
  pitch);
- every DMA trigger (8 group loads + the stores) issues from the otherwise
  idle sync engine's hardware DGE with 4 groups of lookahead, stores
  emitted ahead of the next load on the queue; gpsimd (which cannot touch
  PSUM on TRN2) only loads the w_gcn2 const and does startup memsets;
- the three PSUM evacuations per batch (R1, W2, R2) are spread across
  ACT/DVE by a build-time greedy balancer using measured per-op costs
  (ACT 0.87ns/col + 230, DVE 1.04ns/col + 135), with strict alternation
  for the drain group so its evacuations run in parallel;
- R2 ships raggedly packed by exact column count, per-group mid-kernel and
  per-slot for the last two groups so the final store is small;
- groups are processed in ascending size order, which measures fastest
  (the tensor engine stays continuously fed through the body - matmul
  durations confirm full clock; other orders measured up to 20% slower
  clocks). Steady state is tensor/evac co-paced at ~770ns/slot; the ~8us
  walrus NEFF epilogue (serialized per-engine semaphore zeroing) and
  ~2.5us first-DMA latency are fixed overheads outside kernel control.
"""

import sys

if "/opt/trn_rl_repo" not in sys.path:
    sys.path.insert(0, "/opt/trn_rl_repo")

import ml_dtypes
import numpy as np

import concourse.bass as bass
import concourse.tile as tile
from concourse import bacc, mybir
from concourse.bass_utils import run_bass_kernel_spmd

B, N, H, LAT, OUT = 256, 512, 128, 128, 2
NCORES = 8
BPC = B // NCORES          # batches per core = 32
PT = N // 128              # 4 node tiles max
GRP = 4                    # batches per group (one input / output DMA each)
NGRP = BPC // GRP          # 8 groups per core

F32 = mybir.dt.float32
BF16 = mybir.dt.bfloat16
F8 = mybir.dt.float8e4
AF = mybir.ActivationFunctionType
ALU = mybir.AluOpType
DR = mybir.MatmulPerfMode.DoubleRow
NPBF16 = ml_dtypes.bfloat16
NPF8 = mybir.dt.np(F8)

# power-of-2 prescales (exact; folded back out in the evacuation ops)
EA = 2.0 ** 7              # adjacency
EC = 2.0 ** 5              # L1
ER1 = 2.0 ** 8             # R1 (fp8 intermediate)
ER2 = 2.0 ** 11            # W2 (fp8 intermediate)
ER3 = 2.0 ** 13            # R2 (fp8 output shipped to host)

# group processing order: small first (fast pipeline fill), smallest
# last (fast drain); the big groups stream through the middle where the
# DMA queue and tensor engine are both saturated.
ORDER = (0, 1, 2, 3, 4, 5, 6, 7)
LOOKAHEAD = 4

_CACHE = {}


def _evac_plan(order_slots, cfg):
    """Greedy-assign the PSUM evacuations to ACT/DVE by running-total cost
    (measured-ns models: ACT 0.87/col + 230, DVE 1.04/col + 135)."""
    kts, sges, nouts, kteffs, gno, noff, hasb = cfg
    load = {"act": 0.0, "dve": 0.0}
    assign = {}
    items = []
    for s in order_slots:
        items.append(("r1", s, nouts[s]))
        items.append(("w2", s, kteffs[s] * 128))
        items.append(("r2", s, nouts[s]))
    tail = set(order_slots[-GRP:])
    for ev, s, cols in items:
        if hasb and ev in ("r1", "r2"):
            assign[(ev, s)] = "act"
            load["act"] += 0.87 * cols + 230
            continue
        if ev == "r2" and s in tail:
            # strict alternation so the drain's evacuations run in parallel
            e = ("dve", "act")[s % 2]
        else:
            cost = {"act": 0.87 * cols + 230, "dve": 1.04 * cols + 135}
            e = min(("act", "dve"), key=lambda c: load[c] + cost[c])
        assign[(ev, s)] = e
        load[e] += {"act": 0.87, "dve": 1.04}[e] * cols + \
                   {"act": 230, "dve": 135}[e]
    return assign


def _build(cfg):
    """cfg = (kts, sges, nouts, kteffs, gno, noff, hasb): per-group max tile
    counts and shipped adjacency widths, per-slot exact output widths and
    effective tile counts, per-group ragged output width, per-slot ragged
    output column offset, nonzero-gcn-bias flag."""
    kts, sges, nouts, kteffs, gno, noff, hasb = cfg
    nc = bacc.Bacc("TRN2", target_bir_lowering=False, debug=False,
                   enable_asserts=False, num_devices=NCORES)

    # per-slot tile offset inside its group's merged tensor
    ktoff = [0] * BPC
    gkt = [0] * NGRP
    for g in range(NGRP):
        off = 0
        for bb in range(GRP):
            ktoff[g * GRP + bb] = off
            off += kteffs[g * GRP + bb]
        gkt[g] = off

    agl, r2o = [], []
    for g in range(NGRP):
        sge = sges[g]
        # agl_g[p, toff+u, 0:sge]   = (EA*Adj^T)[u*128+p, i] of its slot
        # agl_g[p, toff+u, sge:]    = (EC*L1)[u*128+p, h]
        agl.append(nc.dram_tensor(f"agl{g}", [128, gkt[g], sge + H], F8,
                                  kind="ExternalInput").ap())
        r2o.append(nc.dram_tensor(f"r2o{g}", [H, gno[g]], F8,
                                  kind="ExternalOutput").ap())
    wg2 = nc.dram_tensor("wg2", [H, H], BF16, kind="ExternalInput").ap()
    if hasb:
        b1s = nc.dram_tensor("b1s", [H, 1], F32, kind="ExternalInput").ap()
        b2s = nc.dram_tensor("b2s", [H, 1], F32, kind="ExternalInput").ap()

    order_slots = [ORDER[j] * GRP + bb for j in range(NGRP)
                   for bb in range(GRP)]
    ev_of = _evac_plan(order_slots, cfg)

    with tile.TileContext(nc) as tc:
        with tc.tile_pool(name="consts", bufs=1) as cpool, \
             tc.tile_pool(name="agl", bufs=7) as agl_pool, \
             tc.tile_pool(name="r1", bufs=3) as r1_pool, \
             tc.tile_pool(name="w2", bufs=3) as w2_pool, \
             tc.tile_pool(name="r2g", bufs=3) as r2g_pool, \
             tc.tile_pool(name="r2t", bufs=4) as r2t_pool, \
             tc.tile_pool(name="psR1", bufs=3, space="PSUM") as psR1_pool, \
             tc.tile_pool(name="psG", bufs=2, space="PSUM") as psG_pool, \
             tc.tile_pool(name="psR2", bufs=3, space="PSUM") as psR2_pool:

            agl_of, r1_of, w2_of, r2_of = {}, {}, {}, {}

            def dma_in(j):
                g = ORDER[j]
                AGL = agl_pool.tile([128, gkt[g], sges[g] + H], F8, tag="agl")
                nc.sync.dma_start(AGL[:], agl[g][:])
                agl_of[g] = AGL

            for j in range(LOOKAHEAD):
                dma_in(j)

            # w_gcn2 const on gpsimd's queue so sync stays free for groups
            WG2 = cpool.tile([H, H], BF16)
            nc.gpsimd.dma_start(WG2[:], wg2[:])
            if hasb:
                B1S = cpool.tile([H, 1], F32)
                nc.gpsimd.dma_start(B1S[:], b1s[:])
                B2S = cpool.tile([H, 1], F32)
                nc.gpsimd.dma_start(B2S[:], b2s[:])

            # stale R1 columns beyond a slot's exact width feed provably
            # cancelled products; memset once so they are finite fp8
            for _z in range(3):
                R1Z = r1_pool.tile([H, N], F8, tag="r1t")
                nc.gpsimd.memset(R1Z[:], 0)

            def adj_pass(psum, lhs3, lbase, AGL, abase, kt, no):
                """psum += sum_u lhs3[:,lbase+u,:]^T @ adj[:,abase+u,:no]."""
                ndr, rem = kt // 2, kt % 2
                for u in range(ndr):
                    nc.tensor.matmul(
                        psum, lhs3[:, lbase + 2 * u:lbase + 2 * u + 2, :],
                        AGL[:, abase + 2 * u:abase + 2 * u + 2, :no],
                        start=(u == 0), stop=(rem == 0 and u == ndr - 1),
                        perf_mode=DR)
                if rem:
                    nc.tensor.matmul(
                        psum, lhs3[:, lbase + kt - 1, :],
                        AGL[:, abase + kt - 1, :no],
                        start=(ndr == 0), stop=True)

            def evac(which, s, dst, src, scale):
                e = ev_of[(which, s)]
                if which == "w2":
                    if e == "act":
                        nc.scalar.activation(dst, src, AF.Copy, scale=scale)
                    else:
                        nc.vector.tensor_scalar_mul(dst, src, scale)
                elif hasb:
                    bias = B1S if which == "r1" else B2S
                    nc.scalar.activation(dst, src, AF.Relu, bias=bias[:],
                                         scale=scale)
                elif e == "act":
                    nc.scalar.activation(dst, src, AF.Relu, scale=scale)
                else:
                    nc.vector.tensor_scalar(dst, src, scale, 0.0,
                                            ALU.mult, ALU.max)

            for t in range(BPC + 4):
                if t % 4 == 0 and t // 4 + LOOKAHEAD < NGRP:
                    dma_in(t // 4 + LOOKAHEAD)

                if t < BPC:
                    # pass1: psR1 = (EA*EC) * L1^T Adj^T  over kteff tiles
                    s = order_slots[t]
                    g = s // GRP
                    no = nouts[s]
                    sge = sges[g]
                    AGL = agl_of[g]
                    psR1 = psR1_pool.tile([H, N], F32, tag="psr1")
                    adj_pass(psR1[:, :no], AGL[:, :, sge:sge + H],
                             ktoff[s], AGL, ktoff[s], kteffs[s], no)
                    R1T = r1_pool.tile([H, N], F8, tag="r1t")
                    evac("r1", s, R1T[:, :no], psR1[:, :no], ER1 / (EA * EC))
                    r1_of[s] = R1T

                if 0 <= t - 2 < BPC:
                    # G: psG[:, u, :] = ER1 * (R1 @ w2) tile u (layout fix)
                    s1 = order_slots[t - 2]
                    kt1 = kteffs[s1]
                    R1T = r1_of.pop(s1)
                    psG = psG_pool.tile([128, PT, H], F32, tag="psg")
                    for u in range(kt1):
                        nc.tensor.matmul(
                            psG[:, u, :], R1T[:, bass.ts(u, 128)],
                            WG2[:], start=True, stop=True)
                    W2T = w2_pool.tile([128, PT, H], F8, tag="w2t")
                    evac("w2", s1, W2T[:, :kt1, :], psG[:, :kt1, :],
                         ER2 / ER1)
                    w2_of[s1] = W2T

                if 0 <= t - 4 < BPC:
                    # pass2 + fp8 R2 evacuation into the ragged group tile
                    s2 = order_slots[t - 4]
                    g2, bb2 = divmod(s2, GRP)
                    no2 = nouts[s2]
                    W2T = w2_of.pop(s2)
                    psR2 = psR2_pool.tile([H, N], F32, tag="psr2")
                    adj_pass(psR2[:, :no2], W2T, 0,
                             agl_of[g2], ktoff[s2], kteffs[s2], no2)
                    lastg = (t - 4) // 4 == NGRP - 1
                    if lastg:
                        # drain-group slots evacuate into per-slot tiles:
                        # a shared tile would serialize the evacuations
                        # (same-tile writers are ordered by Tile) just
                        # when the PE is emitting psR2s back-to-back
                        R2G = r2t_pool.tile([H, N], F8, tag="r2t")
                    else:
                        if bb2 == 0:
                            R2G = r2g_pool.tile([H, gno[g2]], F8,
                                                tag="r2g")
                            r2_of[g2] = R2G
                        R2G = r2_of[g2]
                    o2 = noff[s2]
                    if lastg:
                        if s2 == order_slots[-1] and not hasb and no2 >= 32:
                            hf = no2 // 2
                            sc = ER3 / (EA * ER2)
                            nc.scalar.activation(R2G[:, :hf], psR2[:, :hf],
                                                 AF.Relu, scale=sc)
                            S2T = cpool.tile([H, 256], F8)
                            nc.vector.tensor_scalar(S2T[:, :no2 - hf],
                                                    psR2[:, hf:no2], sc,
                                                    0.0, ALU.mult, ALU.max)
                            nc.sync.dma_start(r2o[g2][:, o2:o2 + hf],
                                              R2G[:, :hf])
                            nc.scalar.dma_start(
                                r2o[g2][:, o2 + hf:o2 + no2],
                                S2T[:, :no2 - hf])
                        else:
                            evac("r2", s2, R2G[:, :no2], psR2[:, :no2],
                                 ER3 / (EA * ER2))
                            nc.sync.dma_start(r2o[g2][:, o2:o2 + no2],
                                              R2G[:, :no2])
                        continue
                    if s2 == order_slots[-1] and not hasb and no2 >= 32:
                        # the very last evacuation+store is the exposed
                        # tail chain: evacuate the two halves into
                        # DIFFERENT tiles (same-tile writers serialize in
                        # Tile) on ACT and DVE, and ship them from two
                        # DMA queues so both chains run fully in parallel
                        hf = no2 // 2
                        sc = ER3 / (EA * ER2)
                        nc.scalar.activation(R2G[:, o2:o2 + hf],
                                             psR2[:, :hf], AF.Relu,
                                             scale=sc)
                        S2T = cpool.tile([H, 256], F8)
                        nc.vector.tensor_scalar(S2T[:, :no2 - hf],
                                                psR2[:, hf:no2], sc, 0.0,
                                                ALU.mult, ALU.max)
                        nc.sync.dma_start(r2o[g2][:, o2:o2 + hf],
                                          R2G[:, o2:o2 + hf])
                        nc.scalar.dma_start(r2o[g2][:, o2 + hf:o2 + no2],
                                            S2T[:, :no2 - hf])
                    else:
                        evac("r2", s2, R2G[:, o2:o2 + no2],
                             psR2[:, :no2], ER3 / (EA * ER2))
                        if (t - 4) // 4 >= NGRP - 2:
                            # drain groups ship per-slot so the final
                            # store overlaps the remaining evacuations
                            nc.sync.dma_start(
                                r2o[g2][:, o2:o2 + no2],
                                R2G[:, o2:o2 + no2])
                        elif bb2 == GRP - 1:
                            nc.sync.dma_start(r2o[g2][:], R2G[:])

    nc.compile()
    return nc


def _get_nc(cfg):
    if cfg not in _CACHE:
        _CACHE[cfg] = _build(cfg)
    return _CACHE[cfg]


def _plan(num_nodes, hasb):
    """Sort batches ascending by node count, deal round-robin to cores."""
    nn_ = np.asarray(num_nodes)
    order = np.argsort(nn_, kind="stable")                  # ascending
    # slot k of core c runs batch order[k*NCORES + c]
    assign = order.reshape(BPC, NCORES)                     # [slot, core]
    # exact per-slot output width
    nn_slot = nn_[assign].max(axis=1)                       # [BPC]
    nouts = tuple(int(min(int(v), N)) for v in nn_slot)
    # effective contraction tile count per slot (zero rows beyond nn)
    kteffs = tuple(-(-no // 128) for no in nouts)
    # per-group shipped adjacency column count (max slot width in group,
    # 32-aligned so the merged tensor's tile pitch keeps the dual-fp8
    # ldweights address restrictions satisfied)
    sges = tuple(int(-(-max(nouts[g * GRP:(g + 1) * GRP]) // 32) * 32)
                 for g in range(NGRP))
    kts = tuple(int(max(kteffs[g * GRP:(g + 1) * GRP]))
                for g in range(NGRP))
    # ragged output packing: per-slot column offset, per-group total
    noff, gno = [0] * BPC, [0] * NGRP
    for g in range(NGRP):
        off = 0
        for bb in range(GRP):
            noff[g * GRP + bb] = off
            off += nouts[g * GRP + bb]
        gno[g] = off
    return assign, (kts, sges, nouts, kteffs, tuple(gno), tuple(noff),
                    bool(hasb))


def _host_prep(z, input_layout, adj_matrix, num_nodes,
               w_gcn1, b_gcn1, w_gcn2, b_gcn2,
               w_noise, b_noise, w_out, b_out):
    f32 = np.float32
    adj = np.asarray(adj_matrix, f32)
    layout = np.asarray(input_layout, f32)
    nn_ = np.asarray(num_nodes)
    m = (np.arange(N)[None, :] < nn_[:, None]).astype(f32)              # [B,N]

    hasb = bool(np.any(np.asarray(b_gcn1)) or np.any(np.asarray(b_gcn2)))
    assign, cfg = _plan(num_nodes, hasb)
    kts, sges, nouts, kteffs, gno, noff, _ = cfg

    # degree of the masked graph incl. self-loops (BLAS gemv), clamp at 1
    degr = np.matmul(adj, m[:, :, None])[:, :, 0] + m                   # [B,N]
    deg = np.maximum(m * degr, 1.0)
    s = (m / np.sqrt(deg)).astype(f32)                                  # [B,N]

    # Adj^T with normalization+mask folded: at[b,j,i] = s_j A[i,j] s_i (+diag)
    at = np.ascontiguousarray(adj.transpose(0, 2, 1))                   # [B,j,i]
    at *= (EA * s)[:, :, None]
    at *= s[:, None, :]
    idx = np.arange(N)
    at[:, idx, idx] += EA * s * s                                       # diag m/deg
    at8 = at.astype(NPF8)

    l1 = (layout @ (EC * np.asarray(w_gcn1, f32))).astype(NPF8)         # [B,N,H]

    ze = np.maximum(np.asarray(z, f32) @ np.asarray(w_noise, f32)
                    + np.asarray(b_noise, f32), 0.0)                    # [B,H]
    wout = np.asarray(w_out, f32)
    cc = (ze @ wout[H:] + np.asarray(b_out, f32)).astype(f32)           # [B,OUT]

    wg2 = np.ascontiguousarray(np.asarray(w_gcn2, f32)).astype(NPBF16)
    wouth = np.ascontiguousarray(wout[:H])                              # host side

    per_core = [{"wg2": wg2} for _ in range(NCORES)]
    if hasb:
        b1sv = (np.asarray(b_gcn1, f32) * ER1).reshape(H, 1).copy()
        b2sv = (np.asarray(b_gcn2, f32) * ER3).reshape(H, 1).copy()
        for c in range(NCORES):
            per_core[c]["b1s"] = b1sv
            per_core[c]["b2s"] = b2sv

    for g in range(NGRP):
        sge = sges[g]
        gkt = sum(kteffs[g * GRP + bb] for bb in range(GRP))
        for c in range(NCORES):
            ab = np.zeros((128, gkt, sge + H), NPF8)
            off = 0
            for bb in range(GRP):
                k = g * GRP + bb
                kte = kteffs[k]
                sj = 128 * kte
                b = int(assign[k, c])
                # at8[b, :sj, :sge] -> [kte,128,sge] -> [128,kte,sge]
                ab[:, off:off + kte, :sge] = (
                    at8[b, :sj, :sge].reshape(kte, 128, sge)
                    .transpose(1, 0, 2))
                ab[:, off:off + kte, sge:] = (
                    l1[b, :sj, :].reshape(kte, 128, H).transpose(1, 0, 2))
                off += kte
            per_core[c][f"agl{g}"] = ab
    return per_core, (cc, m, wouth, assign, cfg)


def _unpack(res, ctx):
    cc, m, wouth, assign, cfg = ctx
    kts, sges, nouts, kteffs, gno, noff, hasb = cfg
    ots = np.zeros((B, N, OUT), np.float32)
    inv_er3 = np.float32(1.0 / ER3)
    for g in range(NGRP):
        for c in range(NCORES):
            r2 = res.results[c][f"r2o{g}"].astype(np.float32)          # [H,gno]
            for bb in range(GRP):
                k = g * GRP + bb
                b = int(assign[k, c])
                no = nouts[k]
                o = noff[k]
                ots[b, :no, :] = (r2[:, o:o + no].T @ wouth) * inv_er3
    out = (ots + cc[:, None, :]) * m[:, :, None]
    return np.ascontiguousarray(out).astype(np.float32)


def kernel(**inputs):
    in_maps, ctx = _host_prep(**inputs)
    nc = _get_nc(ctx[4])
    res = run_bass_kernel_spmd(nc, in_maps, list(range(NCORES)))
    return _unpack(res, ctx)


# revision 47
# speedup vs baseline: 1.0289x; 1.0279x over previous
"""Trainium2 Bass kernel for ConditionalGraphGenerator (GCN message passing).

Contract: kernel(**inputs) takes the FULL unsharded inputs (numpy arrays,
keys as in reference.setup_inputs()) and returns the FULL [256, 512, 2]
float32 output. Internally shards the batch dim across 8 NeuronCores
(pure data parallel, 32 batches per core).

Math (same factorization as the previous revision): the symmetric
normalization and node-validity masking fold into one adjacency on host:
Adj = s.(A+diag(m)).s with s = m*deg^-1/2, so Adj is exactly zero outside
the leading [nn, nn] block (nn = num_nodes). Per batch the device computes
  R1 = relu(Adj @ L1)           L1 = layout@w_gcn1 (host)
  W2 = (R1 @ w_gcn2)            layout-fixing transposing MMs
  R2 = relu(Adj @ W2)
restricted to the leading ceil(nn/128) node tiles and nn output columns.
The output projection, noise path, bias adds, and final mask run on host:
out = m.(R2^T wout[:H] + cc),  cc = relu(z@w_noise+b_noise)@w_out[H:]+b_out.
(gcn biases are zero in the graded inputs; a slower exact bias path exists.)

This revision restructures the schedule around the measured bottlenecks of
the previous one (DMA-trigger sequencer time on the busy engines, clock
p-state resets from tensor-engine gaps, ACT/DVE evacuation imbalance, and
padded-tile DMA traffic):
- adjacency and L1 ship as ONE merged fp8 dram tensor per 4-batch group,
  packed raggedly by each slot's exact tile count (kteff), so a group costs
  one DMA trigger and ships no padded node tiles; compute/evacuation column
  counts are the exact per-slot node counts (only the shipped adjacency
  width is 32-aligned - the dual-fp8 ldweights path needs aligned tile
  pitch);
- every DMA trigger (8 group loads + the stores) issues from the otherwise
  idle sync engine's hardware DGE with 4 groups of lookahead, stores
  emitted ahead of the next load on the queue; gpsimd (which cannot touch
  PSUM on TRN2) only loads the w_gcn2 const and does startup memsets;
- the three PSUM evacuations per batch (R1, W2, R2) are spread across
  ACT/DVE by a build-time greedy balancer using measured per-op costs
  (ACT 0.87ns/col + 230, DVE 1.04ns/col + 135), with strict alternation
  for the drain group so its evacuations run in parallel;
- R2 ships raggedly packed by exact column count, per-group mid-kernel and
  per-slot for the last two groups so the final store is small;
- groups are processed in ascending size order, which measures fastest
  (the tensor engine stays continuously fed through the body - matmul
  durations confirm full clock; other orders measured up to 20% slower
  clocks). Steady state is tensor/evac co-paced at ~770ns/slot; the ~8us
  walrus NEFF epilogue (serialized per-engine semaphore zeroing) and
  ~2.5us first-DMA latency are fixed overheads outside kernel control.
"""

import sys

if "/opt/trn_rl_repo" not in sys.path:
    sys.path.insert(0, "/opt/trn_rl_repo")

import ml_dtypes
import numpy as np

import concourse.bass as bass
import concourse.tile as tile
from concourse import bacc, mybir
from concourse.bass_utils import run_bass_kernel_spmd

B, N, H, LAT, OUT = 256, 512, 128, 128, 2
NCORES = 8
BPC = B // NCORES          # batches per core = 32
PT = N // 128              # 4 node tiles max
GRP = 4                    # batches per group (one input / output DMA each)
NGRP = BPC // GRP          # 8 groups per core

F32 = mybir.dt.float32
BF16 = mybir.dt.bfloat16
F8 = mybir.dt.float8e4
AF = mybir.ActivationFunctionType
ALU = mybir.AluOpType
DR = mybir.MatmulPerfMode.DoubleRow
NPBF16 = ml_dtypes.bfloat16
NPF8 = mybir.dt.np(F8)

# power-of-2 prescales (exact; folded back out in the evacuation ops)
EA = 2.0 ** 7              # adjacency
EC = 2.0 ** 5              # L1
ER1 = 2.0 ** 8             # R1 (fp8 intermediate)
ER2 = 2.0 ** 11            # W2 (fp8 intermediate)
ER3 = 2.0 ** 13            # R2 (fp8 output shipped to host)

# group processing order: small first (fast pipeline fill), smallest
# last (fast drain); the big groups stream through the middle where the
# DMA queue and tensor engine are both saturated.
ORDER = (0, 1, 2, 3, 4, 5, 6, 7)
LOOKAHEAD = 4

_CACHE = {}


def _evac_plan(order_slots, cfg):
    """Greedy-assign the PSUM evacuations to ACT/DVE by running-total cost
    (measured-ns models: ACT 0.87/col + 230, DVE 1.04/col + 135)."""
    kts, sges, nouts, kteffs, gno, noff, hasb = cfg
    load = {"act": 0.0, "dve": 0.0}
    assign = {}
    items = []
    for s in order_slots:
        items.append(("r1", s, nouts[s]))
        items.append(("w2", s, kteffs[s] * 128))
        items.append(("r2", s, nouts[s]))
    tail = set(order_slots[-GRP:])
    for ev, s, cols in items:
        if hasb and ev in ("r1", "r2"):
            assign[(ev, s)] = "act"
            load["act"] += 0.87 * cols + 230
            continue
        if ev == "r2" and s in tail:
            # strict alternation so the drain's evacuations run in parallel
            e = ("dve", "act")[s % 2]
        else:
            cost = {"act": 0.87 * cols + 230, "dve": 1.04 * cols + 135}
            e = min(("act", "dve"), key=lambda c: load[c] + cost[c])
        assign[(ev, s)] = e
        load[e] += {"act": 0.87, "dve": 1.04}[e] * cols + \
                   {"act": 230, "dve": 135}[e]
    return assign


def _build(cfg):
    """cfg = (kts, sges, nouts, kteffs, gno, noff, hasb): per-group max tile
    counts and shipped adjacency widths, per-slot exact output widths and
    effective tile counts, per-group ragged output width, per-slot ragged
    output column offset, nonzero-gcn-bias flag."""
    kts, sges, nouts, kteffs, gno, noff, hasb = cfg
    nc = bacc.Bacc("TRN2", target_bir_lowering=False, debug=False,
                   enable_asserts=False, num_devices=NCORES)

    # per-slot tile offset inside its group's merged tensor
    ktoff = [0] * BPC
    gkt = [0] * NGRP
    for g in range(NGRP):
        off = 0
        for bb in range(GRP):
            ktoff[g * GRP + bb] = off
            off += kteffs[g * GRP + bb]
        gkt[g] = off

    agl, r2o = [], []
    for g in range(NGRP):
        sge = sges[g]
        # agl_g[p, toff+u, 0:sge]   = (EA*Adj^T)[u*128+p, i] of its slot
        # agl_g[p, toff+u, sge:]    = (EC*L1)[u*128+p, h]
        agl.append(nc.dram_tensor(f"agl{g}", [128, gkt[g], sge + H], F8,
                                  kind="ExternalInput").ap())
        r2o.append(nc.dram_tensor(f"r2o{g}", [H, gno[g]], F8,
                                  kind="ExternalOutput").ap())
    wg2 = nc.dram_tensor("wg2", [H, H], BF16, kind="ExternalInput").ap()
    if hasb:
        b1s = nc.dram_tensor("b1s", [H, 1], F32, kind="ExternalInput").ap()
        b2s = nc.dram_tensor("b2s", [H, 1], F32, kind="ExternalInput").ap()

    order_slots = [ORDER[j] * GRP + bb for j in range(NGRP)
                   for bb in range(GRP)]
    ev_of = _evac_plan(order_slots, cfg)

    with tile.TileContext(nc) as tc:
        with tc.tile_pool(name="consts", bufs=1) as cpool, \
             tc.tile_pool(name="agl", bufs=7) as agl_pool, \
             tc.tile_pool(name="r1", bufs=3) as r1_pool, \
             tc.tile_pool(name="w2", bufs=3) as w2_pool, \
             tc.tile_pool(name="r2g", bufs=3) as r2g_pool, \
             tc.tile_pool(name="psR1", bufs=3, space="PSUM") as psR1_pool, \
             tc.tile_pool(name="psG", bufs=2, space="PSUM") as psG_pool, \
             tc.tile_pool(name="psR2", bufs=3, space="PSUM") as psR2_pool:

            agl_of, r1_of, w2_of, r2_of = {}, {}, {}, {}

            def dma_in(j):
                g = ORDER[j]
                AGL = agl_pool.tile([128, gkt[g], sges[g] + H], F8, tag="agl")
                nc.sync.dma_start(AGL[:], agl[g][:])
                agl_of[g] = AGL

            for j in range(LOOKAHEAD):
                dma_in(j)

            # w_gcn2 const on gpsimd's queue so sync stays free for groups
            WG2 = cpool.tile([H, H], BF16)
            nc.gpsimd.dma_start(WG2[:], wg2[:])
            if hasb:
                B1S = cpool.tile([H, 1], F32)
                nc.gpsimd.dma_start(B1S[:], b1s[:])
                B2S = cpool.tile([H, 1], F32)
                nc.gpsimd.dma_start(B2S[:], b2s[:])

            # stale R1 columns beyond a slot's exact width feed provably
            # cancelled products; memset once so they are finite fp8
            for _z in range(3):
                R1Z = r1_pool.tile([H, N], F8, tag="r1t")
                nc.gpsimd.memset(R1Z[:], 0)

            def adj_pass(psum, lhs3, lbase, AGL, abase, kt, no):
                """psum += sum_u lhs3[:,lbase+u,:]^T @ adj[:,abase+u,:no]."""
                ndr, rem = kt // 2, kt % 2
                for u in range(ndr):
                    nc.tensor.matmul(
                        psum, lhs3[:, lbase + 2 * u:lbase + 2 * u + 2, :],
                        AGL[:, abase + 2 * u:abase + 2 * u + 2, :no],
                        start=(u == 0), stop=(rem == 0 and u == ndr - 1),
                        perf_mode=DR)
                if rem:
                    nc.tensor.matmul(
                        psum, lhs3[:, lbase + kt - 1, :],
                        AGL[:, abase + kt - 1, :no],
                        start=(ndr == 0), stop=True)

            def evac(which, s, dst, src, scale):
                e = ev_of[(which, s)]
                if which == "w2":
                    if e == "act":
                        nc.scalar.activation(dst, src, AF.Copy, scale=scale)
                    else:
                        nc.vector.tensor_scalar_mul(dst, src, scale)
                elif hasb:
                    bias = B1S if which == "r1" else B2S
                    nc.scalar.activation(dst, src, AF.Relu, bias=bias[:],
                                         scale=scale)
                elif e == "act":
                    nc.scalar.activation(dst, src, AF.Relu, scale=scale)
                else:
                    nc.vector.tensor_scalar(dst, src, scale, 0.0,
                                            ALU.mult, ALU.max)

            for t in range(BPC + 4):
                if t % 4 == 0 and t // 4 + LOOKAHEAD < NGRP:
                    dma_in(t // 4 + LOOKAHEAD)

                if t < BPC:
                    # pass1: psR1 = (EA*EC) * L1^T Adj^T  over kteff tiles
                    s = order_slots[t]
                    g = s // GRP
                    no = nouts[s]
                    sge = sges[g]
                    AGL = agl_of[g]
                    psR1 = psR1_pool.tile([H, N], F32, tag="psr1")
                    adj_pass(psR1[:, :no], AGL[:, :, sge:sge + H],
                             ktoff[s], AGL, ktoff[s], kteffs[s], no)
                    R1T = r1_pool.tile([H, N], F8, tag="r1t")
                    evac("r1", s, R1T[:, :no], psR1[:, :no], ER1 / (EA * EC))
                    r1_of[s] = R1T

                if 0 <= t - 2 < BPC:
                    # G: psG[:, u, :] = ER1 * (R1 @ w2) tile u (layout fix)
                    s1 = order_slots[t - 2]
                    kt1 = kteffs[s1]
                    R1T = r1_of.pop(s1)
                    psG = psG_pool.tile([128, PT, H], F32, tag="psg")
                    for u in range(kt1):
                        nc.tensor.matmul(
                            psG[:, u, :], R1T[:, bass.ts(u, 128)],
                            WG2[:], start=True, stop=True)
                    W2T = w2_pool.tile([128, PT, H], F8, tag="w2t")
                    evac("w2", s1, W2T[:, :kt1, :], psG[:, :kt1, :],
                         ER2 / ER1)
                    w2_of[s1] = W2T

                if 0 <= t - 4 < BPC:
                    # pass2 + fp8 R2 evacuation into the ragged group tile
                    s2 = order_slots[t - 4]
                    g2, bb2 = divmod(s2, GRP)
                    no2 = nouts[s2]
                    W2T = w2_of.pop(s2)
                    psR2 = psR2_pool.tile([H, N], F32, tag="psr2")
                    adj_pass(psR2[:, :no2], W2T, 0,
                             agl_of[g2], ktoff[s2], kteffs[s2], no2)
                    if bb2 == 0:
                        R2G = r2g_pool.tile([H, gno[g2]], F8, tag="r2g")
                        r2_of[g2] = R2G
                    R2G = r2_of[g2]
                    o2 = noff[s2]
                    if s2 == order_slots[-1] and not hasb and no2 >= 32:
                        # the very last evacuation+store is the exposed
                        # tail chain: evacuate the two halves into
                        # DIFFERENT tiles (same-tile writers serialize in
                        # Tile) on ACT and DVE, and ship them from two
                        # DMA queues so both chains run fully in parallel
                        hf = no2 // 2
                        sc = ER3 / (EA * ER2)
                        nc.scalar.activation(R2G[:, o2:o2 + hf],
                                             psR2[:, :hf], AF.Relu,
                                             scale=sc)
                        S2T = cpool.tile([H, 256], F8)
                        nc.vector.tensor_scalar(S2T[:, :no2 - hf],
                                                psR2[:, hf:no2], sc, 0.0,
                                                ALU.mult, ALU.max)
                        nc.sync.dma_start(r2o[g2][:, o2:o2 + hf],
                                          R2G[:, o2:o2 + hf])
                        nc.scalar.dma_start(r2o[g2][:, o2 + hf:o2 + no2],
                                            S2T[:, :no2 - hf])
                    else:
                        evac("r2", s2, R2G[:, o2:o2 + no2],
                             psR2[:, :no2], ER3 / (EA * ER2))
                        if (t - 4) // 4 >= NGRP - 2:
                            # drain groups ship per-slot so the final
                            # store overlaps the remaining evacuations
                            nc.sync.dma_start(
                                r2o[g2][:, o2:o2 + no2],
                                R2G[:, o2:o2 + no2])
                        elif bb2 == GRP - 1:
                            nc.sync.dma_start(r2o[g2][:], R2G[:])

    nc.compile()
    return nc


def _get_nc(cfg):
    if cfg not in _CACHE:
        _CACHE[cfg] = _build(cfg)
    return _CACHE[cfg]


def _plan(num_nodes, hasb):
    """Sort batches ascending by node count, deal round-robin to cores."""
    nn_ = np.asarray(num_nodes)
    order = np.argsort(nn_, kind="stable")                  # ascending
    # slot k of core c runs batch order[k*NCORES + c]
    assign = order.reshape(BPC, NCORES)                     # [slot, core]
    # exact per-slot output width
    nn_slot = nn_[assign].max(axis=1)                       # [BPC]
    nouts = tuple(int(min(int(v), N)) for v in nn_slot)
    # effective contraction tile count per slot (zero rows beyond nn)
    kteffs = tuple(-(-no // 128) for no in nouts)
    # per-group shipped adjacency column count (max slot width in group,
    # 32-aligned so the merged tensor's tile pitch keeps the dual-fp8
    # ldweights address restrictions satisfied)
    sges = tuple(int(-(-max(nouts[g * GRP:(g + 1) * GRP]) // 32) * 32)
                 for g in range(NGRP))
    kts = tuple(int(max(kteffs[g * GRP:(g + 1) * GRP]))
                for g in range(NGRP))
    # ragged output packing: per-slot column offset, per-group total
    noff, gno = [0] * BPC, [0] * NGRP
    for g in range(NGRP):
        off = 0
        for bb in range(GRP):
            noff[g * GRP + bb] = off
            off += nouts[g * GRP + bb]
        gno[g] = off
    return assign, (kts, sges, nouts, kteffs, tuple(gno), tuple(noff),
                    bool(hasb))


def _host_prep(z, input_layout, adj_matrix, num_nodes,
               w_gcn1, b_gcn1, w_gcn2, b_gcn2,
               w_noise, b_noise, w_out, b_out):
    f32 = np.float32
    adj = np.asarray(adj_matrix, f32)
    layout = np.asarray(input_layout, f32)
    nn_ = np.asarray(num_nodes)
    m = (np.arange(N)[None, :] < nn_[:, None]).astype(f32)              # [B,N]

    hasb = bool(np.any(np.asarray(b_gcn1)) or np.any(np.asarray(b_gcn2)))
    assign, cfg = _plan(num_nodes, hasb)
    kts, sges, nouts, kteffs, gno, noff, _ = cfg

    # degree of the masked graph incl. self-loops (BLAS gemv), clamp at 1
    degr = np.matmul(adj, m[:, :, None])[:, :, 0] + m                   # [B,N]
    deg = np.maximum(m * degr, 1.0)
    s = (m / np.sqrt(deg)).astype(f32)                                  # [B,N]

    # Adj^T with normalization+mask folded: at[b,j,i] = s_j A[i,j] s_i (+diag)
    at = np.ascontiguousarray(adj.transpose(0, 2, 1))                   # [B,j,i]
    at *= (EA * s)[:, :, None]
    at *= s[:, None, :]
    idx = np.arange(N)
    at[:, idx, idx] += EA * s * s                                       # diag m/deg
    at8 = at.astype(NPF8)

    l1 = (layout @ (EC * np.asarray(w_gcn1, f32))).astype(NPF8)         # [B,N,H]

    ze = np.maximum(np.asarray(z, f32) @ np.asarray(w_noise, f32)
                    + np.asarray(b_noise, f32), 0.0)                    # [B,H]
    wout = np.asarray(w_out, f32)
    cc = (ze @ wout[H:] + np.asarray(b_out, f32)).astype(f32)           # [B,OUT]

    wg2 = np.ascontiguousarray(np.asarray(w_gcn2, f32)).astype(NPBF16)
    wouth = np.ascontiguousarray(wout[:H])                              # host side

    per_core = [{"wg2": wg2} for _ in range(NCORES)]
    if hasb:
        b1sv = (np.asarray(b_gcn1, f32) * ER1).reshape(H, 1).copy()
        b2sv = (np.asarray(b_gcn2, f32) * ER3).reshape(H, 1).copy()
        for c in range(NCORES):
            per_core[c]["b1s"] = b1sv
            per_core[c]["b2s"] = b2sv

    for g in range(NGRP):
        sge = sges[g]
        gkt = sum(kteffs[g * GRP + bb] for bb in range(GRP))
        for c in range(NCORES):
            ab = np.zeros((128, gkt, sge + H), NPF8)
            off = 0
            for bb in range(GRP):
                k = g * GRP + bb
                kte = kteffs[k]
                sj = 128 * kte
                b = int(assign[k, c])
                # at8[b, :sj, :sge] -> [kte,128,sge] -> [128,kte,sge]
                ab[:, off:off + kte, :sge] = (
                    at8[b, :sj, :sge].reshape(kte, 128, sge)
                    .transpose(1, 0, 2))
                ab[:, off:off + kte, sge:] = (
                    l1[b, :sj, :].reshape(kte, 128, H).transpose(1, 0, 2))
                off += kte
            per_core[c][f"agl{g}"] = ab
    return per_core, (cc, m, wouth, assign, cfg)


def _unpack(res, ctx):
    cc, m, wouth, assign, cfg = ctx
    kts, sges, nouts, kteffs, gno, noff, hasb = cfg
    ots = np.zeros((B, N, OUT), np.float32)
    inv_er3 = np.float32(1.0 / ER3)
    for g in range(NGRP):
        for c in range(NCORES):
            r2 = res.results[c][f"r2o{g}"].astype(np.float32)          # [H,gno]
            for bb in range(GRP):
                k = g * GRP + bb
                b = int(assign[k, c])
                no = nouts[k]
                o = noff[k]
                ots[b, :no, :] = (r2[:, o:o + no].T @ wouth) * inv_er3
    out = (ots + cc[:, None, :]) * m[:, :, None]
    return np.ascontiguousarray(out).astype(np.float32)


def kernel(**inputs):
    in_maps, ctx = _host_prep(**inputs)
    nc = _get_nc(ctx[4])
    res = run_bass_kernel_spmd(nc, in_maps, list(range(NCORES)))
    return _unpack(res, ctx)
